# revision 1
# baseline (speedup 1.0000x reference)
"""GraphSAGE (5-layer, mean aggr) on 8 Trainium2 NeuronCores via Bass/Tile.

Strategy (matches the sharding hint):
  - Nodes are sharded contiguously across the 8 cores (12500 nodes each);
    each core owns all edges whose *destination* falls in its shard.
  - Per layer, each core computes y = h @ Wl for its own nodes; an AllGather
    replicates y so that every core can gather arbitrary source-neighbor
    rows (the "halo exchange" -- here the halo is effectively global since
    the graph is random).
  - Aggregation: edges are grouped by 128-node destination blocks; for each
    128-edge sub-tile we gather the 128 source rows of y with one slice of a
    block-level indirect DMA, build a one-hot selection matrix S[e, d] =
    (dst_local[e] == d) on the VectorEngine, and accumulate S^T @ G into
    PSUM on the TensorEngine.  The self term (h @ Wr + bl, pre-scaled by
    max(deg,1)) is added with one vector add, then
    h' = relu(inv_deg * psum) on the ScalarEngine.
  - Features travel as bf16 (halves the dominant random-gather traffic);
    all matmul accumulation is fp32 in PSUM.
"""

import sys
import os
import numpy as np

for _p in ("/opt/trn_rl_repo",):
    if _p not in sys.path and os.path.isdir(_p):
        sys.path.insert(0, _p)

import ml_dtypes  # noqa: E402

BF16 = ml_dtypes.bfloat16

# ---------------------------------------------------------------- constants
N_NODES = 100000
N_EDGES = 1600000
HID = 64
N_LAYERS = 5
N_CORES = 8
P = 128


# ---------------------------------------------------------------- host prep
def prep_host(x, edge_index, Wl, bl, Wr, Wfc, bfc, n_nodes, n_cores):
    """Graph partitioning + per-core table construction (pure numpy)."""
    npc = n_nodes // n_cores          # nodes per core
    nblk = (npc + P - 1) // P         # dst blocks per core
    npcp = nblk * P                   # padded nodes per core

    src = edge_index[0].astype(np.int64)
    dst = edge_index[1].astype(np.int64)
    deg = np.bincount(dst, minlength=n_nodes).astype(np.float32)
    degc = np.maximum(deg, 1.0)
    inv = (1.0 / degc).astype(np.float32)

    order = np.argsort(dst, kind="stable")
    ds = dst[order]
    ss = src[order]
    # remap src node id -> padded row id in the all-gathered y table
    ssr = ((ss // npc) * npcp + (ss % npc)).astype(np.int32)

    lcl = ds % npc
    core_of = ds // npc
    gblk = (core_of * nblk + lcl // P).astype(np.int64)
    dloc = (lcl % P).astype(np.float32)

    cnt = np.bincount(gblk, minlength=n_cores * nblk)
    kb = np.ceil(cnt.reshape(n_cores, nblk) / P).astype(np.int64).max(axis=0)
    kb = np.maximum(kb, 1)            # per-block sub-tile count (max over cores)
    off = np.zeros(nblk, np.int64)
    off[1:] = np.cumsum(kb)[:-1]
    kt = int(kb.sum())

    idx = np.zeros((n_cores, P, kt), np.int32)
    dstl = np.full((n_cores, P, kt), 200.0, np.float32)  # 200 => pad (no match)
    starts = np.zeros(n_cores * nblk + 1, np.int64)
    starts[1:] = np.cumsum(cnt)
    for c in range(n_cores):
        for b in range(nblk):
            g = c * nblk + b
            s0, s1 = starts[g], starts[g + 1]
            e = s1 - s0
            if e == 0:
                continue
            j = np.arange(e)
            pp = j % P
            kk = j // P + off[b]
            idx[c, pp, kk] = ssr[s0:s1]
            dstl[c, pp, kk] = dloc[s0:s1]

    # per-core per-block node scales, block-major [P, nblk]
    scl = np.ones((n_cores, P, nblk), np.float32)
    dgc = np.ones((n_cores, P, nblk), np.float32)
    xpad = np.zeros((n_cores, npcp, HID), np.float32)
    for c in range(n_cores):
        nid = c * npc + np.arange(npc)
        pp = np.arange(npc) % P
        bb = np.arange(npc) // P
        scl[c, pp, bb] = inv[nid]
        dgc[c, pp, bb] = degc[nid]
        xpad[c, :npc] = x[c * npc:(c + 1) * npc]

    # weights, SBUF-layout
    nl = Wl.shape[0]
    wl_h = np.zeros((HID, nl * HID), np.float32)   # [f, l*64+f'] = Wl[l,f,f']
    wr_h = np.zeros((HID, nl * HID), np.float32)
    bl_h = np.zeros((P, nl * HID), np.float32)     # replicated across partitions
    for l in range(nl):
        wl_h[:, l * HID:(l + 1) * HID] = Wl[l]
        wr_h[:, l * HID:(l + 1) * HID] = Wr[l]
        bl_h[:, l * HID:(l + 1) * HID] = bl[l][None, :]
    wfct_h = Wfc.reshape(5, HID).T.astype(np.float32)    # [64, 5]
    iota_h = np.broadcast_to(np.arange(P, dtype=np.float32), (P, P)).copy()
    ident_h = np.eye(P, dtype=np.float32)

    ng = npc // 5                      # graphs per core
    gb = (ng + P - 1) // P             # head groups per core

    in_maps = []
    for c in range(n_cores):
        in_maps.append({
            "x_in": xpad[c].astype(np.float32),
            "idx_in": idx[c],
            "dstl_in": dstl[c].astype(np.float32),
            "scl_in": scl[c],
            "dgc_in": dgc[c],
            "iota_in": iota_h.astype(BF16),
            "ident_in": ident_h.astype(BF16),
            "wl_in": wl_h.astype(BF16),
            "wr_in": wr_h.astype(BF16),
            "bl_in": bl_h.astype(BF16),
            "wfct_in": wfct_h.astype(BF16),
        })
    params = dict(kb=[int(v) for v in kb], off=[int(v) for v in off], kt=kt,
                  nblk=nblk, npcp=npcp, npc=npc, ng=ng, gb=gb,
                  bfc=float(np.asarray(bfc).reshape(-1)[0]))
    return in_maps, params


# ---------------------------------------------------------------- program
def build_program(nc, params, n_cores, reps=1, variant='full'):
    import concourse.bass as bass
    import concourse.tile as tile
    from concourse import mybir
    from contextlib import ExitStack

    f32 = mybir.dt.float32
    bf16 = mybir.dt.bfloat16
    i32 = mybir.dt.int32
    AF = mybir.ActivationFunctionType
    OP = mybir.AluOpType

    kb, off, kt = params["kb"], params["off"], params["kt"]
    nblk, npcp, ng, gb = params["nblk"], params["npcp"], params["ng"], params["gb"]
    bfc = params["bfc"]
    nl = N_LAYERS

    x_in = nc.dram_tensor("x_in", [npcp, HID], f32, kind="ExternalInput")
    idx_in = nc.dram_tensor("idx_in", [P, kt], i32, kind="ExternalInput")
    dstl_in = nc.dram_tensor("dstl_in", [P, kt], f32, kind="ExternalInput")
    scl_in = nc.dram_tensor("scl_in", [P, nblk], f32, kind="ExternalInput")
    dgc_in = nc.dram_tensor("dgc_in", [P, nblk], f32, kind="ExternalInput")
    iota_in = nc.dram_tensor("iota_in", [P, P], bf16, kind="ExternalInput")
    ident_in = nc.dram_tensor("ident_in", [P, P], bf16, kind="ExternalInput")
    wl_in = nc.dram_tensor("wl_in", [HID, nl * HID], bf16, kind="ExternalInput")
    wr_in = nc.dram_tensor("wr_in", [HID, nl * HID], bf16, kind="ExternalInput")
    bl_in = nc.dram_tensor("bl_in", [P, nl * HID], bf16, kind="ExternalInput")
    wfct_in = nc.dram_tensor("wfct_in", [HID, 5], bf16, kind="ExternalInput")
    out_t = nc.dram_tensor("out", [P, gb], f32, kind="ExternalOutput")

    groups = [list(range(n_cores))]

    with tile.TileContext(nc) as tc:
        _frees = []
        with ExitStack() as ctx:
            # ---- persistent SBUF (freed LIFO after the pools close)
            idx_sb, _f = tc.tile([P, kt], i32, name="idx_sb"); _frees.append(_f)
            dstl_sb, _f = tc.tile([P, kt], f32, name="dstl_sb"); _frees.append(_f)
            scl_sb, _f = tc.tile([P, nblk], f32, name="scl_sb"); _frees.append(_f)
            dgc_sb, _f = tc.tile([P, nblk], f32, name="dgc_sb"); _frees.append(_f)
            iota_sb, _f = tc.tile([P, P], bf16, name="iota_sb"); _frees.append(_f)
            ident_sb, _f = tc.tile([P, P], bf16, name="ident_sb"); _frees.append(_f)
            wl_sb, _f = tc.tile([HID, nl * HID], bf16, name="wl_sb"); _frees.append(_f)
            wr_sb, _f = tc.tile([HID, nl * HID], bf16, name="wr_sb"); _frees.append(_f)
            bl_sb, _f = tc.tile([P, nl * HID], bf16, name="bl_sb"); _frees.append(_f)
            wfct_sb, _f = tc.tile([HID, 5], bf16, name="wfct_sb"); _frees.append(_f)
            z0_sb, _f = tc.tile([P, nblk * HID], bf16, name="z0_sb"); _frees.append(_f)
            z1_sb, _f = tc.tile([P, nblk * HID], bf16, name="z1_sb"); _frees.append(_f)
            h5t_sb, _f = tc.tile([HID, nblk * P], bf16, name="h5t_sb"); _frees.append(_f)

            nc.sync.dma_start(idx_sb[:], idx_in[:])
            nc.sync.dma_start(dstl_sb[:], dstl_in[:])
            nc.sync.dma_start(scl_sb[:], scl_in[:])
            nc.sync.dma_start(dgc_sb[:], dgc_in[:])
            nc.sync.dma_start(iota_sb[:], iota_in[:])
            nc.sync.dma_start(ident_sb[:], ident_in[:])
            nc.sync.dma_start(wl_sb[:], wl_in[:])
            nc.sync.dma_start(wr_sb[:], wr_in[:])
            nc.sync.dma_start(bl_sb[:], bl_in[:])
            nc.sync.dma_start(wfct_sb[:], wfct_in[:])

            # ---- pools
            dram = ctx.enter_context(tc.tile_pool(name="dram", bufs=1, space="DRAM"))
            sb = ctx.enter_context(tc.tile_pool(name="sb", bufs=3))
            gp = ctx.enter_context(tc.tile_pool(name="gp", bufs=3))
            ps = ctx.enter_context(tc.tile_pool(name="ps", bufs=2, space="PSUM"))

            def transpose_h(hb_ap, dst_slice=None):
                t_ps = ps.tile([HID, P], bf16, tag="tps", name="t_ps")
                nc.tensor.transpose(t_ps[:], hb_ap, ident_sb[:])
                if dst_slice is None:
                    hbt = sb.tile([HID, P], bf16, tag="hbt", name="hbt", bufs=4)
                    nc.vector.tensor_copy(hbt[:], t_ps[:])
                    return hbt
                nc.vector.tensor_copy(dst_slice, t_ps[:])
                return None

            def produce_yz(hbt_ap, lw, b, y_own, zout):
                y_ps = ps.tile([P, HID], f32, tag="yzps", name="y_ps")
                nc.tensor.matmul(y_ps[:], lhsT=hbt_ap,
                                 rhs=wl_sb[:, lw * HID:(lw + 1) * HID],
                                 start=True, stop=True)
                yb = sb.tile([P, HID], bf16, tag="yb", name="yb", bufs=4)
                nc.scalar.copy(yb[:], y_ps[:])
                nc.sync.dma_start(y_own[b * P:(b + 1) * P, :], yb[:])
                z_ps = ps.tile([P, HID], f32, tag="yzps", name="z_ps")
                nc.tensor.matmul(z_ps[:], lhsT=hbt_ap,
                                 rhs=wr_sb[:, lw * HID:(lw + 1) * HID],
                                 start=True, stop=True)
                nc.vector.tensor_tensor(
                    out=z_ps[:], in0=z_ps[:],
                    in1=bl_sb[:, lw * HID:(lw + 1) * HID], op=OP.add)
                nc.scalar.activation(zout[:, b * HID:(b + 1) * HID], z_ps[:],
                                     AF.Copy, scale=dgc_sb[:, b:b + 1])

            # ---- bootstrap: h0 = x -> y0, z0
            for _rep in range(reps):
              y_own = dram.tile([npcp, HID], bf16, tag="yown", name="y_own_b")
              for b in range(nblk):
                  xb = sb.tile([P, HID], bf16, tag="xb", name="xb")
                  nc.gpsimd.dma_start(xb[:], x_in[b * P:(b + 1) * P, :])
                  hbt = transpose_h(xb[:])
                  produce_yz(hbt[:], 0, b, y_own, z0_sb)
              y_full = dram.tile([n_cores * npcp, HID], bf16, tag="yfull",
                                 name="y_full_b")
              if variant == 'no_coll':
                  nc.sync.dma_start(y_full[:npcp, :], y_own[:])
              else:
                  nc.gpsimd.collective_compute(
                      "AllGather", OP.bypass, replica_groups=groups,
                      ins=[y_own.opt()], outs=[y_full.opt()])

              zin, zout = z0_sb, z1_sb
              for l in range(nl):
                  last = l == nl - 1
                  if not last:
                      y_own = dram.tile([npcp, HID], bf16, tag="yown",
                                        name=f"y_own_{l}")
                  for b in range(nblk):
                      k = kb[b]
                      o = off[b]
                      g_ts = []
                      for kk in range(k):
                          g_t = gp.tile([P, HID], bf16, tag="g", name="g_t")
                          g_ts.append(g_t)
                          if variant == 'direct_gather':
                              r0 = ((b * 23 + kk * 7) % 700) * P
                              nc.sync.dma_start(g_t[:], y_full[r0:r0 + P, :])
                          else:
                              nc.gpsimd.indirect_dma_start(
                                  out=g_t[:],
                                  out_offset=None, in_=y_full[:],
                                  in_offset=bass.IndirectOffsetOnAxis(
                                      ap=idx_sb[:, o + kk:o + kk + 1], axis=0))
                      a_ps = ps.tile([P, HID], f32, tag="aps", name="a_ps", bufs=3)
                      nsub = 1 if variant == 'gathers_only' else k
                      for kk in range(nsub):
                          s_t = sb.tile([P, P], bf16, tag="s", name="s_t", bufs=24)
                          nc.vector.tensor_scalar(
                              s_t[:], iota_sb[:], dstl_sb[:, o + kk:o + kk + 1],
                              None, op0=OP.is_equal)
                          nc.tensor.matmul(a_ps[:], lhsT=s_t[:],
                                           rhs=g_ts[kk][:],
                                           start=(kk == 0), stop=(kk == nsub - 1))
                      nc.vector.tensor_tensor(
                          out=a_ps[:], in0=a_ps[:],
                          in1=zin[:, b * HID:(b + 1) * HID], op=OP.add)
                      hb = sb.tile([P, HID], bf16, tag="hb", name="hb", bufs=4)
                      nc.scalar.activation(hb[:], a_ps[:], AF.Relu,
                                           scale=scl_sb[:, b:b + 1])
                      if last:
                          transpose_h(hb[:], dst_slice=h5t_sb[:, b * P:(b + 1) * P])
                      else:
                          hbt = transpose_h(hb[:])
                          produce_yz(hbt[:], l + 1, b, y_own, zout)
                  if not last:
                      y_full = dram.tile([n_cores * npcp, HID], bf16, tag="yfull",
                                         name=f"y_full_{l}")
                      nc.gpsimd.collective_compute(
                          "AllGather", OP.bypass, replica_groups=groups,
                          ins=[y_own.opt()], outs=[y_full.opt()])
                      zin, zout = zout, zin

              # ---- head: out[g] = sigmoid(sum_j h5[5g+j] . wfc_j + bfc)
              hd_ps = ps.tile([P, gb], f32, tag="aps", name="hd_ps", bufs=3)
              for t in range(gb):
                  gcnt = min(P, ng - t * P)
                  for j in range(5):
                      c0 = 5 * t * P + j
                      lhsT = h5t_sb[:, c0:c0 + 5 * gcnt - 4:5]
                      nc.tensor.matmul(hd_ps[:gcnt, t:t + 1], lhsT=lhsT,
                                       rhs=wfct_sb[:, j:j + 1],
                                       start=(j == 0), stop=(j == 4))
              out_sb = sb.tile([P, gb], f32, tag="outsb", name="out_sb")
              bfc_sb = sb.tile([P, 1], f32, tag="bfc", name="bfc_sb")
              nc.vector.memset(bfc_sb[:], bfc)
              nc.scalar.activation(out_sb[:], hd_ps[:], AF.Sigmoid, bias=bfc_sb[:])
              nc.sync.dma_start(out_t[:], out_sb[:])

        for _f in reversed(_frees):
            _f()

    return out_t


def make_nc(params, n_cores, enable_asserts=False, reps=1, variant='full'):
    import concourse.bacc as bacc
    nc = bacc.Bacc("TRN2", target_bir_lowering=False, debug=False,
                   enable_asserts=enable_asserts, num_devices=n_cores)
    build_program(nc, params, n_cores, reps=reps, variant=variant)
    nc.compile()
    return nc


def assemble_output(results, params, n_cores):
    """results: list (per core) of dicts with 'out' [P, gb] f32."""
    ng, gb = params["ng"], params["gb"]
    out = np.zeros((n_cores * ng, 1), np.float32)
    for c in range(n_cores):
        o = np.asarray(results[c]["out"])          # [P, gb]
        flat = o.T.reshape(-1)[:ng]                # graph g = t*P + p
        out[c * ng:(c + 1) * ng, 0] = flat
    return out


# ---------------------------------------------------------------- entry
def kernel(x, edge_index, Wl, bl, Wr, Wfc, bfc):
    from concourse.bass_utils import run_bass_kernel_spmd

    x = np.asarray(x, dtype=np.float32)
    edge_index = np.asarray(edge_index, dtype=np.int32)
    in_maps, params = prep_host(x, edge_index, np.asarray(Wl), np.asarray(bl),
                                np.asarray(Wr), np.asarray(Wfc),
                                np.asarray(bfc), x.shape[0], N_CORES)
    nc = make_nc(params, N_CORES)
    res = run_bass_kernel_spmd(nc, in_maps, core_ids=list(range(N_CORES)))
    return assemble_output(res.results, params, N_CORES)

